# revision 4
# baseline (speedup 1.0000x reference)
"""Trainium2 Bass kernel for the recurrent STP network (nn_Network_20109036880204).

Strategy: tensor-parallel over the output-neuron dim across 8 NeuronCores,
with fp8e4m3 DoubleRow matmuls (K=256 per PE pass -> ~2x tensor throughput).

  - Each core owns a 1024-neuron shard of Wab, stored fp8 (scaled by SW)
    resident in SBUF, pre-packed as 32 k-pair-tiles x [2 x group] blocks.
  - All [B, N] state tensors live in SBUF in "state layout": tile [128, 256]
    with  tile[p, j*32 + b] = state[b, n = c*1024 + j*128 + p].
  - Per step: y = u'*x'*r scaled by SY -> fp8 -> DRAM -> AllGather(8) ->
    y^T in SBUF -> 96 DoubleRow matmuls (32 k-pairs x 3 column groups) ->
    PE transpose of the [32, 1024] result back into state layout -> fused
    DVE update chain (the 1/(SY*SW) scale is folded into dt_tau_syn).
  - THREE-piece pipeline (P0 = j0-1, P1 = j2-3, P2 = j4-7), one output
    column group per piece. Each piece's y is gathered separately as soon
    as its group's K-accumulation stops, so each collective hides under
    the remaining matmuls / the next step's early phases.
"""

import sys

for _p in ("/opt/trn_rl_repo", "/root/.axon_site/_ro/trn_rl_repo"):
    if _p not in sys.path:
        sys.path.append(_p)

import numpy as np
import ml_dtypes

import concourse.bass as bass
import concourse.bacc as bacc
import concourse.mybir as mybir
import concourse.tile as tile
from concourse import bass_utils, masks

# problem constants
NCORES = 8
B = 32
N = 8192
NS = N // NCORES          # 1024 neurons per core
P = 128
J = NS // P               # 8 local K-tiles per core
T = N // P                # 64 K-tiles total
F = J * B                 # 256 = free size of a state tile
NPAIR = T // 2            # 32 k-pair-tiles (K=256 each)

DT = 0.01
USE = 0.03
TAU_FAC = 1.0
TAU_REC = 0.25
C1 = DT / TAU_FAC         # 0.01
C0 = DT * USE / TAU_FAC   # 3e-4
A1 = USE * DT             # 3e-4
C2 = DT / TAU_REC         # 0.04

# fp8 scales: mm_psum = (y*SY) @ (W*SW)^T = mm * SY * SW; folded into dt_syn
SY = 64.0                 # y*64  in [0, ~30]   (e4m3 max 240)
SW = 512.0                # W*512 in [-32, 32]
SCALE = SY * SW           # 2^15

F32 = mybir.dt.float32
F8 = mybir.dt.float8e4
NP_F8 = ml_dtypes.float8_e4m3
MULT = mybir.AluOpType.mult
ADD = mybir.AluOpType.add
MAX = mybir.AluOpType.max
DR = mybir.MatmulPerfMode.DoubleRow

# ---- three pieces: js per piece; one output column group per piece ----
PIECE_JS = [[0, 1], [2, 3], [4, 5, 6, 7]]
NP_ = len(PIECE_JS)
PW = [len(js) * B for js in PIECE_JS]            # [64, 64, 128] y cols/core
G_BOUNDS = [0, 256, 512, 1024]
GW = [G_BOUNDS[g + 1] - G_BOUNDS[g] for g in range(NP_)]   # [256, 256, 512]
GOFF2 = [0, 2 * GW[0], 2 * GW[0] + 2 * GW[1]]    # [0, 512, 1024] in w8 q-block
# state-layout col slices per piece
PSL = []
_acc = 0
for js in PIECE_JS:
    PSL.append(slice(_acc, _acc + len(js) * B))
    _acc += len(js) * B

# k-pair list per piece: within-core (j even, j odd) pairs
PIECE_PAIRS = []
for js in PIECE_JS:
    prs = []
    for i in range(0, len(js), 2):
        for c in range(NCORES):
            prs.append((c * J + js[i], c * J + js[i + 1]))
    PIECE_PAIRS.append(prs)
PAIRS = [pr for prs in PIECE_PAIRS for pr in prs]   # q-order: piece-major
Q_OF_PAIR = {pr: q for q, pr in enumerate(PAIRS)}


def _pair_yoff(pr):
    """(piece, col offset in that piece's yfull) of pair pr's o=0 tile."""
    c, j = divmod(pr[0], J)
    for pi, js in enumerate(PIECE_JS):
        if j in js:
            i = js.index(j)
            return pi, c * PW[pi] + i * B
    raise AssertionError


def build_program(n_steps: int, uni=(None, None, None, None)):
    """Build the SPMD Bass program (identical on all 8 cores)."""
    es_v, ds_v, e_v, dt_v = uni  # uniform values of the const vectors, or None

    nc = bacc.Bacc(
        "TRN2",
        target_bir_lowering=False,
        debug=False,
        num_devices=NCORES,
    )

    w_dram = nc.dram_tensor("w8", [P, NPAIR * 2048], F8, kind="ExternalInput")
    sd = {
        nm: nc.dram_tensor(nm, [P, F], F32, kind="ExternalInput")
        for nm in ["r0", "recs0", "u0", "x0", "ff", "es", "ds", "e", "dt"]
    }
    r_out = nc.dram_tensor("r_out", [P, F], F32, kind="ExternalOutput")

    with tile.TileContext(nc) as tc:
        with (
            tc.tile_pool(name="wpool", bufs=1) as wpool,
            tc.tile_pool(name="cpool", bufs=1) as cpool,
            tc.tile_pool(name="spool", bufs=2) as spool,
            tc.tile_pool(name="wk", bufs=2) as wk,
            tc.tile_pool(name="yp", bufs=2) as yp,
            tc.tile_pool(name="pmm", bufs=2, space="PSUM") as pmm,
            tc.tile_pool(name="pT", bufs=2, space="PSUM") as pT,
            tc.tile_pool(name="dp", bufs=3, space="DRAM") as dp,
        ):
            # ---- resident fp8 weights: 16 DMAs spread across queues ----
            w_sb = wpool.tile([P, NPAIR * 2048], F8, tag="w")
            NW = 16
            CH = NPAIR * 2048 // NW
            for i in range(NW):
                nc.sync.dma_start(
                    w_sb[:, i * CH:(i + 1) * CH],
                    w_dram[:, i * CH:(i + 1) * CH],
                )

            # ---- constants / initial state ----
            ff_sb = cpool.tile([P, F], F32, tag="ff")
            es_sb = cpool.tile([P, F], F32, tag="es")
            ds_sb = cpool.tile([P, F], F32, tag="ds")
            e_sb = cpool.tile([P, F], F32, tag="e")
            dt_sb = cpool.tile([P, F], F32, tag="dt")
            ident = cpool.tile([B, B], F32, tag="ident")
            for t_, nm in [(ff_sb, "ff"), (es_sb, "es"), (ds_sb, "ds"),
                           (e_sb, "e"), (dt_sb, "dt")]:
                nc.scalar.dma_start(t_[:], sd[nm][:])
            masks.make_identity(nc, ident[:])

            r = spool.tile([P, F], F32, tag="r")
            recS = spool.tile([P, F], F32, tag="recS")
            u0_sb = wk.tile([P, F], F32, tag="u0", bufs=1)
            x0_sb = wk.tile([P, F], F32, tag="x0", bufs=1)
            for t_, nm in [(r, "r0"), (recS, "recs0"), (u0_sb, "u0"),
                           (x0_sb, "x0")]:
                nc.scalar.dma_start(t_[:], sd[nm][:])

            V = nc.vector

            # ---- prologue: u1, x1, y0 from initial state ----
            s1 = wk.tile([P, F], F32, tag="t0", bufs=1)
            m = wk.tile([P, F], F32, tag="t1", bufs=1)
            s2 = wk.tile([P, F], F32, tag="t2", bufs=1)
            un = spool.tile([P, F], F32, tag="u")
            V.tensor_scalar(s1[:], u0_sb[:], 1.0 - C1, C0, MULT, ADD)
            V.tensor_mul(m[:], u0_sb[:], r[:])
            V.scalar_tensor_tensor(s2[:], r[:], A1, s1[:], MULT, ADD)
            V.scalar_tensor_tensor(un[:], m[:], -A1, s2[:], MULT, ADD)

            t2p = wk.tile([P, F], F32, tag="t3", bufs=1)
            t3p = wk.tile([P, F], F32, tag="t4", bufs=1)
            s4 = wk.tile([P, F], F32, tag="t5", bufs=1)
            xn = spool.tile([P, F], F32, tag="x")
            V.tensor_mul(t2p[:], x0_sb[:], r[:])
            V.tensor_mul(t3p[:], un[:], t2p[:])
            V.tensor_scalar(s4[:], x0_sb[:], 1.0 - C2, C2, MULT, ADD)
            V.scalar_tensor_tensor(xn[:], t3p[:], -DT, s4[:], MULT, ADD)

            w0 = wk.tile([P, F], F32, tag="t6", bufs=1)
            ys = wk.tile([P, F], F32, tag="t7", bufs=1)
            yh = []
            V.tensor_mul(w0[:], un[:], xn[:])
            V.tensor_mul(ys[:], w0[:], r[:])
            for pi in range(NP_):
                t_ = yp.tile([P, PW[pi]], F8, tag=f"y{pi}", name=f"y{pi}_pro")
                V.tensor_scalar(t_[:], ys[:, PSL[pi]], SY, None, MULT)
                yh.append(t_)

            ag_counter = [0]

            def launch_ag(pi, ytile):
                """store y piece to DRAM, AllGather, DMA gathered tiles back.

                outs ride the ACT hwdge ring (kept clear of bulk work);
                ins ride the SP ring in collective-completion order.
                """
                k = ag_counter[0] = ag_counter[0] + 1
                w_ = PW[pi]
                ydr = dp.tile([P, w_], F8, tag=f"ydr{pi}", name=f"ydr{pi}_{k}")
                nc.scalar.dma_start(ydr[:], ytile[:])
                yall = dp.tile([NCORES, P, w_], F8, tag=f"yall{pi}",
                               name=f"yall{pi}_{k}")
                nc.gpsimd.collective_compute(
                    "AllGather",
                    mybir.AluOpType.bypass,
                    replica_groups=[list(range(NCORES))],
                    ins=[ydr.opt()],
                    outs=[yall.opt()],
                )
                yfull = yp.tile([P, NCORES * w_], F8, tag=f"yfull{pi}",
                                name=f"yfull{pi}_{k}")
                # block c=0 first (tiny DMA) so the next step's first
                # matmuls ungate as early as possible
                nc.sync.dma_start(yfull[:, :w_], yall[0, :, :])
                nc.sync.dma_start(
                    yfull[:, w_:].rearrange("p (c f) -> p c f", c=NCORES - 1),
                    yall[1:, :, :].rearrange("c p f -> p c f"),
                )
                return yfull

            yfulls = [launch_ag(pi, yh[pi]) for pi in range(NP_)]

            def pair_lhst(yfs, q):
                """[128, 2, 32] fp8 stationary AP for k-tile pair q."""
                pi, off = _pair_yoff(PAIRS[q])
                return yfs[pi][:, off:off + 2 * B].rearrange(
                    "p (o b) -> p o b", o=2)

            def pair_rhs(q, g):
                base = q * 2048 + GOFF2[g]
                return w_sb[:, base:base + 2 * GW[g]].rearrange(
                    "p (o n) -> p o n", o=2)

            # ---- main loop ----
            for it in range(n_steps):
                last = it == n_steps - 1

                # precompute (overlaps AG + matmul on DVE)
                A_t = wk.tile([P, F], F32, tag="A", bufs=1)
                B_t = wk.tile([P, F], F32, tag="B", bufs=1)
                C_t = wk.tile([P, F], F32, tag="C", bufs=1)
                D_t = wk.tile([P, F], F32, tag="D", bufs=1)
                rE = wk.tile([P, F], F32, tag="rE", bufs=1)
                if not last:
                    V.tensor_scalar(A_t[:], un[:], 1.0 - C1, C0, MULT, ADD)
                    V.tensor_scalar(B_t[:], un[:], -A1, A1, MULT, ADD)
                    V.tensor_scalar(C_t[:], xn[:], 1.0 - C2, C2, MULT, ADD)
                    V.tensor_scalar(D_t[:], xn[:], DT, None, MULT)
                if e_v is None:
                    V.tensor_mul(rE[:], r[:], e_sb[:])

                # matmuls: one output group per piece, 32 DoubleRow k-pairs
                # per group. Early-piece-sourced pairs first (their gathers
                # landed first); then per group: late-sourced pairs + close.
                pm = [pmm.tile([B, GW[g]], F32,
                               tag=f"mm{g}", name=f"pm{g}_{it}",
                               bufs=(2 if g < 2 else 1))
                      for g in range(NP_)]
                nmm = [0] * NP_

                def emit_group(g, pairs):
                    for pr in pairs:
                        q = Q_OF_PAIR[pr]
                        nc.tensor.matmul(
                            pm[g][:],
                            lhsT=pair_lhst(yfulls, q),
                            rhs=pair_rhs(q, g),
                            start=(nmm[g] == 0),
                            stop=(nmm[g] == NPAIR - 1),
                            perf_mode=DR,
                        )
                        nmm[g] += 1

                def transpose_piece(pi):
                    """PSUM group pi -> state-layout PSUM [128, PW[pi]].

                    Stage copies ride the DVE (vector) so the ACT ring
                    stays clear for the latency-critical y stores.
                    """
                    js = PIECE_JS[pi]
                    mmT_ = pT.tile([P, PW[pi]], F32, tag=f"mmT{pi}", bufs=1,
                                   name=f"mmT{pi}_{it}")
                    stage = wk.tile([B, len(js) * P], F32, tag=f"stage{pi}",
                                    bufs=1, name=f"stage{pi}_{it}")
                    V.tensor_copy(stage[:], pm[pi][:])
                    for k_ in range(len(js)):
                        nc.tensor.transpose(
                            mmT_[:, k_ * B:(k_ + 1) * B],
                            stage[:, k_ * P:(k_ + 1) * P],
                            ident[:],
                        )
                    return mmT_

                # names for per-piece state slices of this iteration
                rec_new = spool.tile([P, F], F32, tag="recfull")
                r_new = spool.tile([P, F], F32, tag="r")
                recS_new = spool.tile([P, F], F32, tag="recS")
                q = spool.tile([P, F], F32, tag="u")
                v = spool.tile([P, F], F32, tag="x")
                newy = [yp.tile([P, PW[pi]], F8, tag=f"yn{pi}",
                                name=f"yn{pi}_{it}") for pi in range(NP_)]

                def ew_piece(pi, mmT_):
                    """DVE chain; y8 computed first, state carries after."""
                    sl = PSL[pi]
                    HF = PW[pi]
                    if ds_v is not None:
                        V.scalar_tensor_tensor(rec_new[:, sl], mmT_[:],
                                               ds_v, recS[:, sl], MULT, ADD)
                    else:
                        tmp = wk.tile([P, HF], F32, tag=f"w0{pi}", bufs=1)
                        V.tensor_mul(tmp[:], mmT_[:], ds_sb[:, sl])
                        V.tensor_add(rec_new[:, sl], tmp[:], recS[:, sl])
                    h_ = wk.tile([P, HF], F32, tag=f"w1{pi}", bufs=1)
                    V.tensor_add(h_[:], rec_new[:, sl], ff_sb[:, sl])
                    dr_ = wk.tile([P, HF], F32, tag=f"w2{pi}", bufs=1)
                    if dt_v is not None:
                        V.tensor_scalar(dr_[:], h_[:], 0.0, dt_v, MAX, MULT)
                    else:
                        V.scalar_tensor_tensor(dr_[:], h_[:], 0.0,
                                               dt_sb[:, sl], MAX, MULT)
                    if e_v is not None:
                        V.scalar_tensor_tensor(r_new[:, sl], r[:, sl], e_v,
                                               dr_[:], MULT, ADD)
                    else:
                        V.tensor_add(r_new[:, sl], dr_[:], rE[:, sl])
                    if last:
                        return None
                    m1_ = wk.tile([P, HF], F32, tag=f"w3{pi}", bufs=1)
                    V.tensor_mul(m1_[:], B_t[:, sl], r_new[:, sl])
                    V.tensor_add(q[:, sl], m1_[:], A_t[:, sl])
                    tt_ = wk.tile([P, HF], F32, tag=f"w4{pi}", bufs=1)
                    V.tensor_mul(tt_[:], r_new[:, sl], q[:, sl])
                    s2_ = wk.tile([P, HF], F32, tag=f"w5{pi}", bufs=1)
                    V.tensor_mul(s2_[:], D_t[:, sl], tt_[:])
                    V.scalar_tensor_tensor(v[:, sl], s2_[:], -1.0, C_t[:, sl],
                                           MULT, ADD)
                    ynew = newy[pi]
                    # y8 = (SY * tt_) * v   (cast to fp8e4 on write)
                    V.scalar_tensor_tensor(ynew[:], tt_[:], SY, v[:, sl],
                                           MULT, MULT)
                    # state carry AFTER y8 so the AG launches sooner
                    if es_v is not None:
                        V.tensor_scalar(recS_new[:, sl], rec_new[:, sl],
                                        es_v, None, MULT)
                    else:
                        V.tensor_mul(recS_new[:, sl], rec_new[:, sl],
                                     es_sb[:, sl])
                    return ynew

                # early-sourced fronts: pieces 0 and 1 pairs over all groups
                for sp in range(NP_ - 1):
                    for g in range(NP_):
                        emit_group(g, PIECE_PAIRS[sp])
                # late-sourced pairs: close the groups one piece at a time
                nxt = [None] * NP_
                for g in range(NP_):
                    emit_group(g, PIECE_PAIRS[NP_ - 1])
                    mmT_ = transpose_piece(g)
                    yn = ew_piece(g, mmT_)
                    if not last:
                        nxt[g] = launch_ag(g, yn)

                if not last:
                    yfulls = nxt
                    un, xn, recS = q, v, recS_new
                r = r_new

            # ---- epilogue ----
            for qi in range(4):
                nc.sync.dma_start(
                    r_out[32 * qi:32 * (qi + 1), :],
                    r[32 * qi:32 * (qi + 1), :],
                )

    nc.compile()
    return nc


# ---------------------------------------------------------------------------
# host-side data marshalling
# ---------------------------------------------------------------------------

def _shard_state(v, c):
    """[B, N] float array -> core c state tile [128, 256] (f32)."""
    vs = np.asarray(v, np.float32)[:, c * NS:(c + 1) * NS]      # [32, 1024]
    return np.ascontiguousarray(
        vs.reshape(B, J, P).transpose(2, 1, 0).reshape(P, F)
    )


def _shard_vec(v, c):
    """[N] float vector -> replicated core c tile [128, 256] (f32)."""
    vs = np.asarray(v, np.float32)[c * NS:(c + 1) * NS].reshape(J, P)  # [j, p]
    t = vs.T[:, :, None]                                        # [p, j, 1]
    return np.ascontiguousarray(np.broadcast_to(t, (P, J, B)).reshape(P, F))


def _shard_w8(Wab, c):
    """Wab [N, N] -> core c fp8 weight pack [P, NPAIR*2048].

    Layout: [p, (pair q, group g, slot o, n')] =
            Wab[c*1024 + G_BOUNDS[g] + n', 128*t_o + p] * SW
    """
    Wl = np.asarray(Wab, np.float32)[c * NS:(c + 1) * NS, :] * SW  # [1024, 8192]
    out = np.empty((P, NPAIR, 2048), dtype=NP_F8)
    for q, (t0, t1) in enumerate(PAIRS):
        for g in range(NP_):
            lo, hi = G_BOUNDS[g], G_BOUNDS[g + 1]
            wg = hi - lo
            for o, t in enumerate((t0, t1)):
                blk = Wl[lo:hi, t * P:(t + 1) * P]              # [wg, 128]
                out[:, q, GOFF2[g] + o * wg:GOFF2[g] + (o + 1) * wg] = (
                    blk.T.astype(NP_F8))
    return np.ascontiguousarray(out.reshape(P, -1))


def _unshard_out(tiles):
    """list of 8 [128, 256] tiles -> [32, 8192] f32."""
    out = np.empty((B, N), np.float32)
    for c, tl in enumerate(tiles):
        out[:, c * NS:(c + 1) * NS] = (
            np.asarray(tl, np.float32).reshape(P, J, B).transpose(2, 1, 0)
            .reshape(B, NS)
        )
    return out


def make_in_maps(rates, rec_input, ff_input, Wab, u_stp, x_stp,
                 exp_dt_tau, dt_tau, exp_dt_tau_syn, dt_tau_syn):
    recs_full = (np.asarray(exp_dt_tau_syn, np.float32)[None, :]
                 * np.asarray(rec_input, np.float32))
    ds_scaled = np.asarray(dt_tau_syn, np.float32) / SCALE
    in_maps = []
    for c in range(NCORES):
        in_maps.append({
            "w8": _shard_w8(Wab, c),
            "r0": _shard_state(rates, c),
            "recs0": _shard_state(recs_full, c),
            "u0": _shard_state(u_stp, c),
            "x0": _shard_state(x_stp, c),
            "ff": _shard_state(ff_input, c),
            "es": _shard_vec(exp_dt_tau_syn, c),
            "ds": _shard_vec(ds_scaled, c),
            "e": _shard_vec(exp_dt_tau, c),
            "dt": _shard_vec(dt_tau, c),
        })
    return in_maps


_PROGRAM_CACHE = {}


def _uniform_val(v):
    v = np.asarray(v, np.float32)
    return float(v.flat[0]) if np.all(v == v.flat[0]) else None


def _get_program(n_steps, uni):
    key = (n_steps, uni)
    if key not in _PROGRAM_CACHE:
        _PROGRAM_CACHE[key] = build_program(n_steps, uni=uni)
    return _PROGRAM_CACHE[key]


def run(trace=False, tmpdir=None, **inputs):
    n_steps = int(inputs.pop("n_steps"))
    uni = (_uniform_val(inputs["exp_dt_tau_syn"]),
           _uniform_val(np.asarray(inputs["dt_tau_syn"], np.float32) / SCALE),
           _uniform_val(inputs["exp_dt_tau"]),
           _uniform_val(inputs["dt_tau"]))
    nc = _get_program(n_steps, uni)
    in_maps = make_in_maps(**inputs)
    res = bass_utils.run_bass_kernel_spmd(
        nc, in_maps, core_ids=list(range(NCORES)), trace=trace, tmpdir=tmpdir
    )
    out = _unshard_out([m["r_out"] for m in res.results])
    return out, res


def kernel(**inputs):
    out, _ = run(**inputs)
    return out


# revision 10
# speedup vs baseline: 1.0847x; 1.0847x over previous
"""Trainium2 Bass kernel for the recurrent STP network (nn_Network_20109036880204).

Strategy: tensor-parallel over the output-neuron dim across 8 NeuronCores,
with fp8e4m3 DoubleRow matmuls (K=256 per PE pass -> ~2x tensor throughput).

  - Each core owns a 1024-neuron shard of Wab, stored fp8 (scaled by SW)
    resident in SBUF, pre-packed as 32 k-pair-tiles x [2 x group] blocks.
  - All [B, N] state tensors live in SBUF in "state layout": tile [128, 256]
    with  tile[p, j*32 + b] = state[b, n = c*1024 + j*128 + p].
  - Per step: y = u'*x'*r scaled by SY -> fp8 -> DRAM -> AllGather(8) ->
    y^T in SBUF -> 96 DoubleRow matmuls (32 k-pairs x 3 column groups) ->
    PE transpose of the [32, 1024] result back into state layout -> fused
    DVE update chain (the 1/(SY*SW) scale is folded into dt_tau_syn).
  - THREE-piece pipeline (P0 = j0-1, P1 = j2-3, P2 = j4-7), one output
    column group per piece. Each piece's y is gathered separately as soon
    as its group's K-accumulation stops, so each collective hides under
    the remaining matmuls / the next step's early phases.
"""

import sys

for _p in ("/opt/trn_rl_repo", "/root/.axon_site/_ro/trn_rl_repo"):
    if _p not in sys.path:
        sys.path.append(_p)

import numpy as np
import ml_dtypes

import concourse.bass as bass
import concourse.bacc as bacc
import concourse.mybir as mybir
import concourse.tile as tile
from concourse import bass_utils, masks

# problem constants
NCORES = 8
B = 32
N = 8192
NS = N // NCORES          # 1024 neurons per core
P = 128
J = NS // P               # 8 local K-tiles per core
T = N // P                # 64 K-tiles total
F = J * B                 # 256 = free size of a state tile
NPAIR = T // 2            # 32 k-pair-tiles (K=256 each)

DT = 0.01
USE = 0.03
TAU_FAC = 1.0
TAU_REC = 0.25
C1 = DT / TAU_FAC         # 0.01
C0 = DT * USE / TAU_FAC   # 3e-4
A1 = USE * DT             # 3e-4
C2 = DT / TAU_REC         # 0.04

# fp8 scales: mm_psum = (y*SY) @ (W*SW)^T = mm * SY * SW; folded into dt_syn
SY = 64.0                 # y*64  in [0, ~30]   (e4m3 max 240)
SW = 512.0                # W*512 in [-32, 32]
SCALE = SY * SW           # 2^15

F32 = mybir.dt.float32
F8 = mybir.dt.float8e4
NP_F8 = ml_dtypes.float8_e4m3
MULT = mybir.AluOpType.mult
ADD = mybir.AluOpType.add
MAX = mybir.AluOpType.max
DR = mybir.MatmulPerfMode.DoubleRow

# ---- two pieces: js per piece; one output column group per piece ----
PIECE_JS = [[0, 1, 2, 3], [4, 5, 6, 7]]
NP_ = len(PIECE_JS)
PW = [len(js) * B for js in PIECE_JS]            # [128, 128] y cols/core
G_BOUNDS = [0, 512, 1024]
GW = [G_BOUNDS[g + 1] - G_BOUNDS[g] for g in range(NP_)]   # [512, 512]
GOFF2 = [0] + list(np.cumsum([2 * w for w in GW]))[:-1]    # [0, 1024]
# state-layout col slices per piece
PSL = []
_acc = 0
for js in PIECE_JS:
    PSL.append(slice(_acc, _acc + len(js) * B))
    _acc += len(js) * B

# k-pair list per piece: within-core (j even, j odd) pairs
PIECE_PAIRS = []
for js in PIECE_JS:
    prs = []
    for i in range(0, len(js), 2):
        for c in range(NCORES):
            prs.append((c * J + js[i], c * J + js[i + 1]))
    PIECE_PAIRS.append(prs)
PAIRS = [pr for prs in PIECE_PAIRS for pr in prs]   # q-order: piece-major
Q_OF_PAIR = {pr: q for q, pr in enumerate(PAIRS)}


def _pair_yoff(pr):
    """(piece, col offset in that piece's yfull) of pair pr's o=0 tile."""
    c, j = divmod(pr[0], J)
    for pi, js in enumerate(PIECE_JS):
        if j in js:
            i = js.index(j)
            return pi, c * PW[pi] + i * B
    raise AssertionError


def build_program(n_steps: int, uni=(None, None, None, None)):
    """Build the SPMD Bass program (identical on all 8 cores)."""
    es_v, ds_v, e_v, dt_v = uni  # uniform values of the const vectors, or None

    nc = bacc.Bacc(
        "TRN2",
        target_bir_lowering=False,
        debug=False,
        num_devices=NCORES,
    )

    w_dram = nc.dram_tensor("w8", [P, NPAIR * 2048], F8, kind="ExternalInput")
    sd = {
        nm: nc.dram_tensor(nm, [P, F], F32, kind="ExternalInput")
        for nm in ["r0", "recs0", "u0", "x0", "ff", "es", "ds", "e", "dt"]
    }
    r_out = nc.dram_tensor("r_out", [P, F], F32, kind="ExternalOutput")

    with tile.TileContext(nc) as tc:
        with (
            tc.tile_pool(name="wpool", bufs=1) as wpool,
            tc.tile_pool(name="cpool", bufs=1) as cpool,
            tc.tile_pool(name="spool", bufs=2) as spool,
            tc.tile_pool(name="wk", bufs=2) as wk,
            tc.tile_pool(name="yp", bufs=2) as yp,
            tc.tile_pool(name="pmm", bufs=2, space="PSUM") as pmm,
            tc.tile_pool(name="pT", bufs=2, space="PSUM") as pT,
            tc.tile_pool(name="dp", bufs=3, space="DRAM") as dp,
        ):
            # ---- resident fp8 weights: 16 DMAs spread across queues ----
            w_sb = wpool.tile([P, NPAIR * 2048], F8, tag="w")
            NW = 16
            CH = NPAIR * 2048 // NW
            for i in range(NW):
                nc.sync.dma_start(
                    w_sb[:, i * CH:(i + 1) * CH],
                    w_dram[:, i * CH:(i + 1) * CH],
                )

            # ---- constants / initial state ----
            ff_sb = cpool.tile([P, F], F32, tag="ff")
            es_sb = cpool.tile([P, F], F32, tag="es")
            ds_sb = cpool.tile([P, F], F32, tag="ds")
            e_sb = cpool.tile([P, F], F32, tag="e")
            dt_sb = cpool.tile([P, F], F32, tag="dt")
            ident = cpool.tile([B, B], F32, tag="ident")
            ident16 = cpool.tile([B, B], mybir.dt.float16, tag="ident16")
            for t_, nm in [(ff_sb, "ff"), (es_sb, "es"), (ds_sb, "ds"),
                           (e_sb, "e"), (dt_sb, "dt")]:
                nc.scalar.dma_start(t_[:], sd[nm][:])
            masks.make_identity(nc, ident[:])
            nc.vector.tensor_copy(ident16[:], ident[:])

            r = spool.tile([P, F], F32, tag="r")
            recS = spool.tile([P, F], F32, tag="recS")
            u0_sb = wk.tile([P, F], F32, tag="u0", bufs=1)
            x0_sb = wk.tile([P, F], F32, tag="x0", bufs=1)
            for t_, nm in [(r, "r0"), (recS, "recs0"), (u0_sb, "u0"),
                           (x0_sb, "x0")]:
                nc.scalar.dma_start(t_[:], sd[nm][:])

            V = nc.vector

            # ---- prologue: u1, x1, y0 from initial state ----
            s1 = wk.tile([P, F], F32, tag="t0", bufs=1)
            m = wk.tile([P, F], F32, tag="t1", bufs=1)
            s2 = wk.tile([P, F], F32, tag="t2", bufs=1)
            un = spool.tile([P, F], F32, tag="u")
            V.tensor_scalar(s1[:], u0_sb[:], 1.0 - C1, C0, MULT, ADD)
            V.tensor_mul(m[:], u0_sb[:], r[:])
            V.scalar_tensor_tensor(s2[:], r[:], A1, s1[:], MULT, ADD)
            V.scalar_tensor_tensor(un[:], m[:], -A1, s2[:], MULT, ADD)

            t2p = wk.tile([P, F], F32, tag="t3", bufs=1)
            t3p = wk.tile([P, F], F32, tag="t4", bufs=1)
            s4 = wk.tile([P, F], F32, tag="t5", bufs=1)
            xn = spool.tile([P, F], F32, tag="x")
            V.tensor_mul(t2p[:], x0_sb[:], r[:])
            V.tensor_mul(t3p[:], un[:], t2p[:])
            V.tensor_scalar(s4[:], x0_sb[:], 1.0 - C2, C2, MULT, ADD)
            V.scalar_tensor_tensor(xn[:], t3p[:], -DT, s4[:], MULT, ADD)

            w0 = wk.tile([P, F], F32, tag="t6", bufs=1)
            ys = wk.tile([P, F], F32, tag="t7", bufs=1)
            yh = []
            V.tensor_mul(w0[:], un[:], xn[:])
            V.tensor_mul(ys[:], w0[:], r[:])
            for pi in range(NP_):
                t_ = yp.tile([P, PW[pi]], F8, tag=f"y{pi}", name=f"y{pi}_pro")
                V.tensor_scalar(t_[:], ys[:, PSL[pi]], SY, None, MULT)
                yh.append(t_)

            ag_counter = [0]

            def launch_ag(pi, ytile):
                """store y piece to DRAM, AllGather, DMA gathered tiles back.

                outs ride the ACT hwdge ring (kept clear of bulk work);
                ins ride the SP ring in collective-completion order.
                """
                k = ag_counter[0] = ag_counter[0] + 1
                w_ = PW[pi]
                ydr = dp.tile([P, w_], F8, tag=f"ydr{pi}", name=f"ydr{pi}_{k}")
                nc.scalar.dma_start(ydr[:], ytile[:])
                yall = dp.tile([NCORES, P, w_], F8, tag=f"yall{pi}",
                               name=f"yall{pi}_{k}")
                nc.gpsimd.collective_compute(
                    "AllGather",
                    mybir.AluOpType.bypass,
                    replica_groups=[list(range(NCORES))],
                    ins=[ydr.opt()],
                    outs=[yall.opt()],
                )
                yfull = yp.tile([P, NCORES * w_], F8, tag=f"yfull{pi}",
                                name=f"yfull{pi}_{k}")
                # block c=0 first (tiny DMA) so the next step's first
                # matmuls ungate as early as possible
                nc.sync.dma_start(yfull[:, :w_], yall[0, :, :])
                nc.sync.dma_start(
                    yfull[:, w_:].rearrange("p (c f) -> p c f", c=NCORES - 1),
                    yall[1:, :, :].rearrange("c p f -> p c f"),
                )
                return yfull

            yfulls = [launch_ag(pi, yh[pi]) for pi in range(NP_)]

            def pair_lhst(yfs, q):
                """[128, 2, 32] fp8 stationary AP for k-tile pair q."""
                pi, off = _pair_yoff(PAIRS[q])
                return yfs[pi][:, off:off + 2 * B].rearrange(
                    "p (o b) -> p o b", o=2)

            def pair_rhs(q, g):
                base = q * 2048 + GOFF2[g]
                return w_sb[:, base:base + 2 * GW[g]].rearrange(
                    "p (o n) -> p o n", o=2)

            # ---- main loop ----
            for it in range(n_steps):
                last = it == n_steps - 1

                # precompute (overlaps AG + matmul on DVE)
                A_t = wk.tile([P, F], F32, tag="A", bufs=1)
                B_t = wk.tile([P, F], F32, tag="B", bufs=1)
                C_t = wk.tile([P, F], F32, tag="C", bufs=1)
                D_t = wk.tile([P, F], F32, tag="D", bufs=1)
                rE = wk.tile([P, F], F32, tag="rE", bufs=1)
                if not last:
                    V.tensor_scalar(A_t[:], un[:], 1.0 - C1, C0, MULT, ADD)
                    V.tensor_scalar(B_t[:], un[:], -A1, A1, MULT, ADD)
                    V.tensor_scalar(C_t[:], xn[:], 1.0 - C2, C2, MULT, ADD)
                    V.tensor_scalar(D_t[:], xn[:], DT, None, MULT)
                if e_v is None:
                    V.tensor_mul(rE[:], r[:], e_sb[:])

                # matmuls: one output group per piece, 32 DoubleRow k-pairs
                # per group. Early-piece-sourced pairs first (their gathers
                # landed first); then per group: late-sourced pairs + close.
                pm = [pmm.tile([B, GW[g]], F32,
                               tag=f"mm{g}", name=f"pm{g}_{it}",
                               bufs=(2 if g < 2 else 1))
                      for g in range(NP_)]
                nmm = [0] * NP_

                def emit_group(g, pairs):
                    for pr in pairs:
                        q = Q_OF_PAIR[pr]
                        nc.tensor.matmul(
                            pm[g][:],
                            lhsT=pair_lhst(yfulls, q),
                            rhs=pair_rhs(q, g),
                            start=(nmm[g] == 0),
                            stop=(nmm[g] == NPAIR - 1),
                            perf_mode=DR,
                        )
                        nmm[g] += 1

                def transpose_piece(pi):
                    """PSUM group pi -> state-layout PSUM [128, PW[pi]].

                    Stage copies ride the DVE (vector) so the ACT ring
                    stays clear for the latency-critical y stores.
                    """
                    js = PIECE_JS[pi]
                    mmT_ = pT.tile([P, PW[pi]], mybir.dt.float16,
                                   tag=f"mmT{pi}", bufs=1,
                                   name=f"mmT{pi}_{it}")
                    stage = wk.tile([B, len(js) * P], mybir.dt.float16,
                                    tag=f"stage{pi}",
                                    bufs=1, name=f"stage{pi}_{it}")
                    V.tensor_copy(stage[:], pm[pi][:])
                    for k_ in range(len(js)):
                        nc.tensor.transpose(
                            mmT_[:, k_ * B:(k_ + 1) * B],
                            stage[:, k_ * P:(k_ + 1) * P],
                            ident16[:],
                        )
                    return mmT_

                # names for per-piece state slices of this iteration
                rec_new = spool.tile([P, F], F32, tag="recfull")
                r_new = spool.tile([P, F], F32, tag="r")
                recS_new = spool.tile([P, F], F32, tag="recS")
                q = spool.tile([P, F], F32, tag="u")
                v = spool.tile([P, F], F32, tag="x")
                newy = [yp.tile([P, PW[pi]], F8, tag=f"yn{pi}",
                                name=f"yn{pi}_{it}") for pi in range(NP_)]

                def ew_piece(pi, mmT_):
                    """DVE chain; y8 computed first, state carries after."""
                    sl = PSL[pi]
                    HF = PW[pi]
                    if ds_v is not None:
                        V.scalar_tensor_tensor(rec_new[:, sl], mmT_[:],
                                               ds_v, recS[:, sl], MULT, ADD)
                    else:
                        tmp = wk.tile([P, HF], F32, tag=f"w0{pi}", bufs=1)
                        V.tensor_mul(tmp[:], mmT_[:], ds_sb[:, sl])
                        V.tensor_add(rec_new[:, sl], tmp[:], recS[:, sl])
                    h_ = wk.tile([P, HF], F32, tag=f"w1{pi}", bufs=1)
                    V.tensor_add(h_[:], rec_new[:, sl], ff_sb[:, sl])
                    dr_ = wk.tile([P, HF], F32, tag=f"w2{pi}", bufs=1)
                    if dt_v is not None:
                        V.tensor_scalar(dr_[:], h_[:], 0.0, dt_v, MAX, MULT)
                    else:
                        V.scalar_tensor_tensor(dr_[:], h_[:], 0.0,
                                               dt_sb[:, sl], MAX, MULT)
                    if e_v is not None:
                        V.scalar_tensor_tensor(r_new[:, sl], r[:, sl], e_v,
                                               dr_[:], MULT, ADD)
                    else:
                        V.tensor_add(r_new[:, sl], dr_[:], rE[:, sl])
                    if last:
                        return None
                    m1_ = wk.tile([P, HF], F32, tag=f"w3{pi}", bufs=1)
                    V.tensor_mul(m1_[:], B_t[:, sl], r_new[:, sl])
                    V.tensor_add(q[:, sl], m1_[:], A_t[:, sl])
                    tt_ = wk.tile([P, HF], F32, tag=f"w4{pi}", bufs=1)
                    V.tensor_mul(tt_[:], r_new[:, sl], q[:, sl])
                    s2_ = wk.tile([P, HF], F32, tag=f"w5{pi}", bufs=1)
                    V.tensor_mul(s2_[:], D_t[:, sl], tt_[:])
                    V.scalar_tensor_tensor(v[:, sl], s2_[:], -1.0, C_t[:, sl],
                                           MULT, ADD)
                    ynew = newy[pi]
                    # y8 = (SY * tt_) * v   (cast to fp8e4 on write)
                    V.scalar_tensor_tensor(ynew[:], tt_[:], SY, v[:, sl],
                                           MULT, MULT)
                    # state carry AFTER y8 so the AG launches sooner
                    if es_v is not None:
                        V.tensor_scalar(recS_new[:, sl], rec_new[:, sl],
                                        es_v, None, MULT)
                    else:
                        V.tensor_mul(recS_new[:, sl], rec_new[:, sl],
                                     es_sb[:, sl])
                    return ynew

                # early-sourced fronts: pieces 0 and 1 pairs over all groups
                for sp in range(NP_ - 1):
                    for g in range(NP_):
                        emit_group(g, PIECE_PAIRS[sp])
                # late-sourced pairs: close the groups one piece at a time
                nxt = [None] * NP_
                for g in range(NP_):
                    emit_group(g, PIECE_PAIRS[NP_ - 1])
                    mmT_ = transpose_piece(g)
                    yn = ew_piece(g, mmT_)
                    if not last:
                        nxt[g] = launch_ag(g, yn)

                if not last:
                    yfulls = nxt
                    un, xn, recS = q, v, recS_new
                r = r_new

            # ---- epilogue ----
            for qi in range(4):
                nc.sync.dma_start(
                    r_out[32 * qi:32 * (qi + 1), :],
                    r[32 * qi:32 * (qi + 1), :],
                )

    nc.compile()
    return nc


# ---------------------------------------------------------------------------
# host-side data marshalling
# ---------------------------------------------------------------------------

def _shard_state(v, c):
    """[B, N] float array -> core c state tile [128, 256] (f32)."""
    vs = np.asarray(v, np.float32)[:, c * NS:(c + 1) * NS]      # [32, 1024]
    return np.ascontiguousarray(
        vs.reshape(B, J, P).transpose(2, 1, 0).reshape(P, F)
    )


def _shard_vec(v, c):
    """[N] float vector -> replicated core c tile [128, 256] (f32)."""
    vs = np.asarray(v, np.float32)[c * NS:(c + 1) * NS].reshape(J, P)  # [j, p]
    t = vs.T[:, :, None]                                        # [p, j, 1]
    return np.ascontiguousarray(np.broadcast_to(t, (P, J, B)).reshape(P, F))


def _shard_w8(Wab, c):
    """Wab [N, N] -> core c fp8 weight pack [P, NPAIR*2048].

    Layout: [p, (pair q, group g, slot o, n')] =
            Wab[c*1024 + G_BOUNDS[g] + n', 128*t_o + p] * SW
    """
    Wl = np.asarray(Wab, np.float32)[c * NS:(c + 1) * NS, :] * SW  # [1024, 8192]
    out = np.empty((P, NPAIR, 2048), dtype=NP_F8)
    for q, (t0, t1) in enumerate(PAIRS):
        for g in range(NP_):
            lo, hi = G_BOUNDS[g], G_BOUNDS[g + 1]
            wg = hi - lo
            for o, t in enumerate((t0, t1)):
                blk = Wl[lo:hi, t * P:(t + 1) * P]              # [wg, 128]
                out[:, q, GOFF2[g] + o * wg:GOFF2[g] + (o + 1) * wg] = (
                    blk.T.astype(NP_F8))
    return np.ascontiguousarray(out.reshape(P, -1))


def _unshard_out(tiles):
    """list of 8 [128, 256] tiles -> [32, 8192] f32."""
    out = np.empty((B, N), np.float32)
    for c, tl in enumerate(tiles):
        out[:, c * NS:(c + 1) * NS] = (
            np.asarray(tl, np.float32).reshape(P, J, B).transpose(2, 1, 0)
            .reshape(B, NS)
        )
    return out


def make_in_maps(rates, rec_input, ff_input, Wab, u_stp, x_stp,
                 exp_dt_tau, dt_tau, exp_dt_tau_syn, dt_tau_syn):
    recs_full = (np.asarray(exp_dt_tau_syn, np.float32)[None, :]
                 * np.asarray(rec_input, np.float32))
    ds_scaled = np.asarray(dt_tau_syn, np.float32) / SCALE
    in_maps = []
    for c in range(NCORES):
        in_maps.append({
            "w8": _shard_w8(Wab, c),
            "r0": _shard_state(rates, c),
            "recs0": _shard_state(recs_full, c),
            "u0": _shard_state(u_stp, c),
            "x0": _shard_state(x_stp, c),
            "ff": _shard_state(ff_input, c),
            "es": _shard_vec(exp_dt_tau_syn, c),
            "ds": _shard_vec(ds_scaled, c),
            "e": _shard_vec(exp_dt_tau, c),
            "dt": _shard_vec(dt_tau, c),
        })
    return in_maps


_PROGRAM_CACHE = {}


def _uniform_val(v):
    v = np.asarray(v, np.float32)
    return float(v.flat[0]) if np.all(v == v.flat[0]) else None


def _get_program(n_steps, uni):
    key = (n_steps, uni)
    if key not in _PROGRAM_CACHE:
        _PROGRAM_CACHE[key] = build_program(n_steps, uni=uni)
    return _PROGRAM_CACHE[key]


def run(trace=False, tmpdir=None, **inputs):
    n_steps = int(inputs.pop("n_steps"))
    uni = (_uniform_val(inputs["exp_dt_tau_syn"]),
           _uniform_val(np.asarray(inputs["dt_tau_syn"], np.float32) / SCALE),
           _uniform_val(inputs["exp_dt_tau"]),
           _uniform_val(inputs["dt_tau"]))
    nc = _get_program(n_steps, uni)
    in_maps = make_in_maps(**inputs)
    res = bass_utils.run_bass_kernel_spmd(
        nc, in_maps, core_ids=list(range(NCORES)), trace=trace, tmpdir=tmpdir
    )
    out = _unshard_out([m["r_out"] for m in res.results])
    return out, res


def kernel(**inputs):
    out, _ = run(**inputs)
    return out
